# revision 30
# baseline (speedup 1.0000x reference)
"""Causal single-head attention (B=4, T=2048, E=1024, D=128) on 8 TRN2 cores.

Sharding: core c = (b, h) with b = c // 2, h = c % 2. Core h of batch b owns
query blocks {2s + (1-h) : s=0..3} (256 queries each; h=0 odd, h=1 even
blocks). Token PERMUTATION trick: each core's xt (and rope tables) are
uploaded with tokens reordered within every 512-token chunk so the core's own
query-256 comes first. One identical program then serves all cores:

  slot s: queries = xt[:, s, :, 0:256] (own half), keys = all chunks
  [0, 512(s+1)) in permuted order, processed as 2(s+1) pairs of two 128-key
  chunks. Pair 2s (own half) gets the structural diagonal mask; pair 2s+1
  (other half) gets an uploaded all-ones (h=0: other half precedes queries)
  or all-zeros (h=1: other half follows) mask.

vs baseline (104987 ns):
  - bf16 activations/weights (DMA 17.9 MB -> ~7 MB, rel err ~5e-3 << 2e-2)
  - no xq / q-table / mask-pack uploads (sliced or affine_select on device)
  - V natural via XBAR DMA transpose (kills 16 PE transposes + 16 ACT copies)
  - paired 512-wide exp (20 ACT ops vs 40), single mask mul per pair
  - PE warmup matmuls during input DMA (p-state ramp 0.65/1.2/2.4 GHz)
  - flattened S/DAV software pipeline (PE never waits on exp)
  - reciprocal_approx_fast + rope copies moved off the critical engines
"""

import sys

for _p in ("/opt/trn_rl_repo",):
    if _p not in sys.path:
        sys.path.insert(0, _p)

import numpy as np
import ml_dtypes

try:
    import antenv.axon_hooks  # noqa: F401
except Exception:
    import types as _types

    _m = _types.ModuleType("antenv.axon_hooks")
    _m.set_axon_ntff_profile_hook = lambda h: None
    _m.get_axon_ntff_profile_hook = lambda: None
    sys.modules.setdefault("antenv.axon_hooks", _m)

import concourse.bacc as bacc
import concourse.mybir as mybir
import concourse.tile as tile
from concourse.bass_utils import run_bass_kernel_spmd

F32 = mybir.dt.float32
BF16 = mybir.dt.bfloat16
BF16_NP = ml_dtypes.bfloat16

B, T, E, D = 4, 2048, 1024, 128
THETA = 10000.0
SCALE = 1.0 / np.sqrt(np.float32(D))
N_CORES = 8
N_EC = E // 128
N_TC = T // 512
BLK = 256
N_SLOT = 4


def _build_nc(dbg=False):
    nc = bacc.Bacc(None, target_bir_lowering=False, debug=False)

    if dbg:
        kT_o = nc.dram_tensor("kT_o", [D, T], BF16, kind="ExternalOutput")
        qT_o = nc.dram_tensor("qT_o", [D, N_SLOT, BLK], BF16, kind="ExternalOutput")
        den_o = nc.dram_tensor("den_o", [128, N_SLOT, BLK], F32, kind="ExternalOutput")
        vn_o = nc.dram_tensor("vn_o", [128, T // 128, D], BF16, kind="ExternalOutput")

    xt_d = nc.dram_tensor("xt", [128, N_TC, N_EC, 512], BF16, kind="ExternalInput")
    wk_d = nc.dram_tensor("wk", [128, N_EC, D], BF16, kind="ExternalInput")
    wv_d = nc.dram_tensor("wv", [128, N_EC, D], BF16, kind="ExternalInput")
    wq_d = nc.dram_tensor("wq", [128, N_EC, D], BF16, kind="ExternalInput")
    ctab_d = nc.dram_tensor("ctab", [D, T], BF16, kind="ExternalInput")
    stab_d = nc.dram_tensor("stab", [D, T], BF16, kind="ExternalInput")
    lastmask_d = nc.dram_tensor("lastmask", [128, 512], BF16, kind="ExternalInput")
    out_d = nc.dram_tensor("out", [D, N_SLOT, BLK], F32, kind="ExternalOutput")

    with tile.TileContext(nc) as tc:
        with (
            tc.tile_pool(name="const", bufs=1) as const,
            tc.tile_pool(name="persist", bufs=1) as persist,
            tc.tile_pool(name="work", bufs=2) as work,
            tc.tile_pool(name="pp", bufs=1, space="PSUM") as pp,
            tc.tile_pool(name="pq", bufs=1, space="PSUM") as pqp,
            tc.tile_pool(name="ps", bufs=3, space="PSUM") as ps,
            tc.tile_pool(name="pa", bufs=1, space="PSUM") as pa,
        ):
            # ---- device-built constants (no DMA dependency) ----
            ones = const.tile([128, 128], BF16)
            nc.gpsimd.memset(ones, 1.0)
            scratch = const.tile([128, 512], BF16)
            nc.gpsimd.memset(scratch, 0.0)
            # diagonal mask for pair 2s: [:, 0:256] chunk keeps y>=x,
            # [:, 256:512] chunk keeps y-x>=128
            mask = const.tile([128, 512], BF16)
            nc.gpsimd.memset(mask, 1.0)
            nc.gpsimd.affine_select(
                out=mask[:, 0:BLK], in_=mask[:, 0:BLK],
                compare_op=mybir.AluOpType.is_ge, fill=0.0,
                base=0, channel_multiplier=-1, pattern=[[1, BLK]],
            )
            nc.gpsimd.affine_select(
                out=mask[:, BLK:2 * BLK], in_=mask[:, BLK:2 * BLK],
                compare_op=mybir.AluOpType.is_ge, fill=0.0,
                base=-128, channel_multiplier=-1, pattern=[[1, BLK]],
            )

            # ---- input DMAs ----
            # sync queue: wk, wv, then xt per-tc (K/V critical path)
            w_sb = {}
            for name, dram, eng in (("k", wk_d, nc.sync), ("v", wv_d, nc.sync),
                                    ("q", wq_d, nc.scalar)):
                t = const.tile([128, N_EC, D], BF16, tag=f"w_{name}")
                eng.dma_start(out=t, in_=dram[:])
                w_sb[name] = t
            xt = persist.tile([128, N_TC, N_EC, 512], BF16)
            for tci in range(N_TC):
                nc.sync.dma_start(out=xt[:, tci, 0:4], in_=xt_d[:, tci, 0:4])
                nc.sync.dma_start(out=xt[:, tci, 4:8], in_=xt_d[:, tci, 4:8])
            # scalar queue: ctab, stab (needed by rope of tc0), lastmask
            ctab_sb = const.tile([D, T], BF16)
            nc.scalar.dma_start(out=ctab_sb, in_=ctab_d[:])
            stab_sb = const.tile([D, T], BF16)
            nc.scalar.dma_start(out=stab_sb, in_=stab_d[:])
            lastmask = const.tile([128, 512], BF16)
            nc.gpsimd.dma_start(out=lastmask, in_=lastmask_d[:])

            # persistent activation tensors
            kT = persist.tile([D, T], BF16)
            qT = persist.tile([D, N_SLOT, BLK], BF16)
            v_nat = persist.tile([128, T // 128, D], BF16)

            # q-table gather: slot s tables = permuted-ctab cols [512s, 512s+256)
            qctab = const.tile([D, N_SLOT * BLK], BF16)
            qstab = const.tile([D, N_SLOT * BLK], BF16)
            for s in range(N_SLOT):
                cs = slice(s * 512, s * 512 + BLK)
                ds = slice(s * BLK, (s + 1) * BLK)
                nc.gpsimd.dma_start(out=qctab[:, ds], in_=ctab_sb[:, cs])
                nc.gpsimd.dma_start(out=qstab[:, ds], in_=stab_sb[:, cs])

            # ---- PE warmup: ramp p-state while inputs stream in ----
            pwarm = ps.tile([128, 512], F32, tag="s", name="pwarm")
            for _ in range(7):
                nc.tensor.matmul(pwarm, ones, scratch, start=True, stop=True)

            qT_flat = qT.rearrange("p s b -> p (s b)")

            def kv_chunk(tci):
                cs = slice(tci * 512, (tci + 1) * 512)
                psk = pp.tile([128, 512], F32, tag="psk", name=f"psk{tci}")
                for ec in range(N_EC):
                    nc.tensor.matmul(psk, w_sb["k"][:, ec, :], xt[:, tci, ec, :],
                                     start=ec == 0, stop=ec == N_EC - 1)
                # V natural directly: out[t,d] = sum_e x[e,t] wv[e,d]
                # (xt 128-token chunk stationary, wv moving)
                psvT = pp.tile([128, 4, 128], F32, tag="psvT", name=f"psvT{tci}")
                for j in range(4):
                    for ec in range(N_EC):
                        nc.tensor.matmul(
                            psvT[:, j, :],
                            xt[:, tci, ec, j * 128:(j + 1) * 128],
                            w_sb["v"][:, ec, :],
                            start=ec == 0, stop=ec == N_EC - 1)
                # rope K: ACT copy psum->sbuf, gpsimd pair-swap, DVE combine
                raw = work.tile([128, 512], F32, tag="raw")
                nc.scalar.copy(raw, psk)
                sw = work.tile([128, 512], F32, tag="sw")
                s2 = raw.rearrange("(a b) f -> a b f", b=2)
                d2 = sw.rearrange("(a b) f -> a b f", b=2)
                nc.gpsimd.dma_start(out=d2[:, 0, :], in_=s2[:, 1, :])
                nc.gpsimd.dma_start(out=d2[:, 1, :], in_=s2[:, 0, :])
                t1 = work.tile([128, 512], F32, tag="ropeA")
                nc.vector.tensor_mul(t1, raw, ctab_sb[:, cs])
                t2 = work.tile([128, 512], F32, tag="ropeB")
                nc.vector.tensor_mul(t2, sw, stab_sb[:, cs])
                nc.vector.tensor_add(kT[:, cs], t1, t2)
                # evict V chunk to SBUF (bf16) on ACT
                vslice = v_nat[:, tci * 4:(tci + 1) * 4, :]
                nc.scalar.copy(vslice, psvT)

            def q_slot(s):
                qs_ = slice(s * BLK, (s + 1) * BLK)
                psq = pqp.tile([128, BLK], F32, tag="psq", name=f"psq{s}")
                for ec in range(N_EC):
                    nc.tensor.matmul(psq, w_sb["q"][:, ec, :],
                                     xt[:, s, ec, 0:BLK],
                                     start=ec == 0, stop=ec == N_EC - 1)
                qraw = work.tile([128, BLK], F32, tag="qraw")
                nc.scalar.copy(qraw, psq)
                qsw = work.tile([128, BLK], F32, tag="qsw")
                qs2 = qraw.rearrange("(a b) f -> a b f", b=2)
                qd2 = qsw.rearrange("(a b) f -> a b f", b=2)
                nc.gpsimd.dma_start(out=qd2[:, 0, :], in_=qs2[:, 1, :])
                nc.gpsimd.dma_start(out=qd2[:, 1, :], in_=qs2[:, 0, :])
                qt1 = work.tile([128, BLK], F32, tag="qropeA")
                nc.vector.tensor_mul(qt1, qraw, qctab[:, qs_])
                qt2 = work.tile([128, BLK], F32, tag="qropeB")
                nc.vector.tensor_mul(qt2, qsw, qstab[:, qs_])
                nc.vector.tensor_add(qT_flat[:, qs_], qt1, qt2)

            # ---- attention slot: software-pipelined S/exp vs D/AV ----
            def emit_S(s, p):
                pss = ps.tile([128, 512], F32, tag="s", name=f"pss{s}_{p}")
                ks = 256 * p
                nc.tensor.matmul(pss[:, 0:BLK], kT[:, ks:ks + 128],
                                 qT[:, s, :], start=True, stop=True)
                nc.tensor.matmul(pss[:, BLK:2 * BLK], kT[:, ks + 128:ks + 256],
                                 qT[:, s, :], start=True, stop=True)
                pT = work.tile([128, 512], BF16, tag="pT", bufs=4)
                nc.scalar.activation(out=pT, in_=pss,
                                     func=mybir.ActivationFunctionType.Exp,
                                     scale=float(SCALE))
                if p == 2 * s:  # own half: structural diagonal mask
                    nc.vector.tensor_mul(pT, pT, mask)
                elif p == 2 * s + 1:  # other half: per-core ones/zeros
                    nc.vector.tensor_mul(pT, pT, lastmask)
                return pT

            pacc = {}

            def emit_DAV(s, p, pT):
                np_ = 2 * (s + 1)
                if p == 0:
                    pacc_av = pa.tile([128, BLK], F32, tag="pacc_av", name=f"pav{s}")
                    pacc_d = pa.tile([128, BLK], F32, tag="pacc_d", name=f"pd{s}")
                    pacc[s] = (pacc_av, pacc_d)
                pacc_av, pacc_d = pacc[s]
                st, sp = p == 0, p == np_ - 1
                nc.tensor.matmul(pacc_d, ones, pT[:, 0:BLK], start=st, stop=False)
                nc.tensor.matmul(pacc_d, ones, pT[:, BLK:2 * BLK],
                                 start=False, stop=sp)
                nc.tensor.matmul(pacc_av, v_nat[:, 2 * p, :], pT[:, 0:BLK],
                                 start=st, stop=False)
                nc.tensor.matmul(pacc_av, v_nat[:, 2 * p + 1, :],
                                 pT[:, BLK:2 * BLK], start=False, stop=sp)
                if sp:  # slot done -> normalize + store
                    if dbg:
                        dcp = work.tile([128, BLK], F32, tag="dcp", name=f"dcp{s}")
                        nc.vector.tensor_copy(dcp, pacc_d)
                        nc.scalar.dma_start(out=den_o[:, s], in_=dcp)
                    recip = work.tile([128, BLK], F32, tag="recip")
                    nc.vector.reciprocal_approx_fast(recip, pacc_d)
                    oT = work.tile([128, BLK], F32, tag="oT")
                    nc.vector.tensor_mul(oT, pacc_av, recip)
                    nc.sync.dma_start(out=out_d[:, s], in_=oT)

            from collections import deque

            def attn_slot(s):
                pend = deque()
                for p in range(2 * (s + 1)):
                    pT = emit_S(s, p)
                    pend.append((s, p, pT))
                    if len(pend) > 2:
                        emit_DAV(*pend.popleft())
                while pend:
                    emit_DAV(*pend.popleft())

            # ---- schedule: attention slots fill the xt2/xt3 DMA windows ----
            kv_chunk(0)
            kv_chunk(1)
            q_slot(1)
            attn_slot(1)
            kv_chunk(2)
            q_slot(2)
            attn_slot(2)
            kv_chunk(3)
            q_slot(0)
            attn_slot(0)
            q_slot(3)
            attn_slot(3)

            if dbg:
                nc.scalar.dma_start(out=kT_o[:], in_=kT)
                nc.scalar.dma_start(out=qT_o[:], in_=qT)
                nc.scalar.dma_start(out=vn_o[:], in_=v_nat)

    nc.compile()
    return nc


_NC = None


def _get_nc():
    global _NC
    if _NC is None:
        _NC = _build_nc()
    return _NC


def _perm(h):
    """Permuted token order: within each 512-chunk, own 256 first."""
    own_off = 256 * (1 - h)  # h=0 owns odd blocks (cols 256:512 of chunk)
    idx = np.empty(T, dtype=np.int64)
    for s in range(N_TC):
        base = 512 * s
        idx[base:base + 256] = base + own_off + np.arange(256)
        idx[base + 256:base + 512] = base + (256 - own_off) + np.arange(256)
    return idx


def _host_prep(embedding_word, w_Q, w_K, w_V):
    x = np.asarray(embedding_word, dtype=np.float32)

    def pack_w(w):
        wt = np.asarray(w, dtype=np.float32).T.astype(BF16_NP)  # [E, D]
        return np.ascontiguousarray(wt.reshape(N_EC, 128, D).transpose(1, 0, 2))

    wq_p, wk_p, wv_p = pack_w(w_Q), pack_w(w_K), pack_w(w_V)

    j = np.arange(D // 2, dtype=np.float64)
    freqs = 1.0 / THETA ** (2.0 * j / D)
    t = np.arange(T, dtype=np.float64)
    ang = np.outer(freqs, t)
    cos = np.cos(ang)
    sin = np.sin(ang)
    ctab = np.repeat(cos, 2, axis=0).astype(BF16_NP)
    stab = np.empty((D, T), dtype=BF16_NP)
    stab[0::2] = -sin
    stab[1::2] = sin

    perms = {h: _perm(h) for h in (0, 1)}
    tabs = {h: (np.ascontiguousarray(ctab[:, perms[h]]),
                np.ascontiguousarray(stab[:, perms[h]])) for h in (0, 1)}
    lastmasks = {0: np.ones((128, 512), dtype=BF16_NP),
                 1: np.zeros((128, 512), dtype=BF16_NP)}

    xt_bh = {}
    for b in range(B):
        xTf = x[b].T.astype(BF16_NP)  # [E, T]
        for h in (0, 1):
            xT = xTf[:, perms[h]]
            xt_bh[b, h] = np.ascontiguousarray(
                xT.reshape(N_EC, 128, N_TC, 512).transpose(1, 2, 0, 3))

    in_maps = []
    for c in range(N_CORES):
        b, h = c // 2, c % 2
        in_maps.append({
            "xt": xt_bh[b, h],
            "wq": wq_p, "wk": wk_p, "wv": wv_p,
            "ctab": tabs[h][0], "stab": tabs[h][1],
            "lastmask": lastmasks[h],
        })
    return in_maps


def _assemble(results):
    out = np.empty((B, T, D), dtype=np.float32)
    for c in range(N_CORES):
        b, h = c // 2, c % 2
        o = results[c]["out"]  # [D, 4, 256] d-major; slot s = block 2s+(1-h)
        for s in range(N_SLOT):
            jb = 2 * s + (1 - h)
            out[b, jb * BLK:(jb + 1) * BLK, :] = o[:, s, :].T
    return out


def run(inputs, trace=False, tmpdir=None):
    nc = _get_nc()
    in_maps = _host_prep(**inputs)
    res = run_bass_kernel_spmd(nc, in_maps, list(range(N_CORES)),
                               trace=trace, tmpdir=tmpdir)
    return _assemble(res.results), res


def kernel(embedding_word, w_Q, w_K, w_V):
    out, _ = run(dict(embedding_word=embedding_word, w_Q=w_Q, w_K=w_K, w_V=w_V))
    return out


# revision 32
# speedup vs baseline: 1.1502x; 1.1502x over previous
"""Causal single-head attention (B=4, T=2048, E=1024, D=128) on 8 TRN2 cores.

Sharding: core c = (b, h) with b = c // 2, h = c % 2. Core h of batch b owns
query blocks {2s + (1-h) : s=0..3} (256 queries each; h=0 odd, h=1 even
blocks). Token PERMUTATION trick: each core's xt (and rope tables) are
uploaded with tokens reordered within every 512-token chunk so the core's own
query-256 comes first. One identical program then serves all cores:

  slot s: queries = xt[:, s, :, 0:256] (own half), keys = all chunks
  [0, 512(s+1)) in permuted order, processed as 2(s+1) pairs of two 128-key
  chunks. Pair 2s (own half) gets the structural diagonal mask; pair 2s+1
  (other half) gets an uploaded all-ones (h=0: other half precedes queries)
  or all-zeros (h=1: other half follows) mask.

vs baseline (104987 ns):
  - bf16 activations/weights (DMA 17.9 MB -> ~7 MB, rel err ~5e-3 << 2e-2)
  - no xq / q-table / mask-pack uploads (sliced or affine_select on device)
  - V natural via XBAR DMA transpose (kills 16 PE transposes + 16 ACT copies)
  - paired 512-wide exp (20 ACT ops vs 40), single mask mul per pair
  - PE warmup matmuls during input DMA (p-state ramp 0.65/1.2/2.4 GHz)
  - flattened S/DAV software pipeline (PE never waits on exp)
  - reciprocal_approx_fast + rope copies moved off the critical engines
"""

import sys

for _p in ("/opt/trn_rl_repo",):
    if _p not in sys.path:
        sys.path.insert(0, _p)

import numpy as np
import ml_dtypes

try:
    import antenv.axon_hooks  # noqa: F401
except Exception:
    import types as _types

    _m = _types.ModuleType("antenv.axon_hooks")
    _m.set_axon_ntff_profile_hook = lambda h: None
    _m.get_axon_ntff_profile_hook = lambda: None
    sys.modules.setdefault("antenv.axon_hooks", _m)

import concourse.bacc as bacc
import concourse.mybir as mybir
import concourse.tile as tile
from concourse.bass_utils import run_bass_kernel_spmd

F32 = mybir.dt.float32
BF16 = mybir.dt.bfloat16
BF16_NP = ml_dtypes.bfloat16

B, T, E, D = 4, 2048, 1024, 128
THETA = 10000.0
SCALE = 1.0 / np.sqrt(np.float32(D))
N_CORES = 8
N_EC = E // 128
N_TC = T // 512
BLK = 256
N_SLOT = 4


def _build_nc(dbg=False):
    nc = bacc.Bacc(None, target_bir_lowering=False, debug=False)

    if dbg:
        kT_o = nc.dram_tensor("kT_o", [D, T], BF16, kind="ExternalOutput")
        qT_o = nc.dram_tensor("qT_o", [D, N_SLOT, BLK], BF16, kind="ExternalOutput")
        den_o = nc.dram_tensor("den_o", [128, N_SLOT, BLK], F32, kind="ExternalOutput")
        vn_o = nc.dram_tensor("vn_o", [128, T // 128, D], BF16, kind="ExternalOutput")

    xt_d = nc.dram_tensor("xt", [128, N_TC, N_EC, 512], BF16, kind="ExternalInput")
    wk_d = nc.dram_tensor("wk", [128, N_EC, D], BF16, kind="ExternalInput")
    wv_d = nc.dram_tensor("wv", [128, N_EC, D], BF16, kind="ExternalInput")
    wq_d = nc.dram_tensor("wq", [128, N_EC, D], BF16, kind="ExternalInput")
    ctab_d = nc.dram_tensor("ctab", [D, T], BF16, kind="ExternalInput")
    stab_d = nc.dram_tensor("stab", [D, T], BF16, kind="ExternalInput")
    lastmask_d = nc.dram_tensor("lastmask", [128, 512], BF16, kind="ExternalInput")
    out_d = nc.dram_tensor("out", [D, N_SLOT, BLK], F32, kind="ExternalOutput")

    with tile.TileContext(nc) as tc:
        with (
            tc.tile_pool(name="const", bufs=1) as const,
            tc.tile_pool(name="persist", bufs=1) as persist,
            tc.tile_pool(name="work", bufs=2) as work,
            tc.tile_pool(name="pp", bufs=1, space="PSUM") as pp,
            tc.tile_pool(name="pq", bufs=1, space="PSUM") as pqp,
            tc.tile_pool(name="ps", bufs=3, space="PSUM") as ps,
            tc.tile_pool(name="pa", bufs=1, space="PSUM") as pa,
        ):
            # ---- device-built constants (no DMA dependency) ----
            ones = const.tile([128, 128], BF16)
            nc.gpsimd.memset(ones, 1.0)
            scratch = const.tile([128, 512], BF16)
            nc.gpsimd.memset(scratch, 0.0)
            # diagonal mask for pair 2s: [:, 0:256] chunk keeps y>=x,
            # [:, 256:512] chunk keeps y-x>=128
            mask = const.tile([128, 512], BF16)
            nc.gpsimd.memset(mask, 1.0)
            nc.gpsimd.affine_select(
                out=mask[:, 0:BLK], in_=mask[:, 0:BLK],
                compare_op=mybir.AluOpType.is_ge, fill=0.0,
                base=0, channel_multiplier=-1, pattern=[[1, BLK]],
            )
            nc.gpsimd.affine_select(
                out=mask[:, BLK:2 * BLK], in_=mask[:, BLK:2 * BLK],
                compare_op=mybir.AluOpType.is_ge, fill=0.0,
                base=-128, channel_multiplier=-1, pattern=[[1, BLK]],
            )

            # ---- input DMAs ----
            # sync queue: wk, wv, then xt per-tc (K/V critical path)
            w_sb = {}
            for name, dram, eng in (("k", wk_d, nc.sync), ("v", wv_d, nc.sync),
                                    ("q", wq_d, nc.scalar)):
                t = const.tile([128, N_EC, D], BF16, tag=f"w_{name}")
                eng.dma_start(out=t, in_=dram[:])
                w_sb[name] = t
            xt = persist.tile([128, N_TC, N_EC, 512], BF16)
            for tci in range(N_TC):
                for e4 in range(0, 8, 2):
                    nc.sync.dma_start(out=xt[:, tci, e4:e4 + 2],
                                      in_=xt_d[:, tci, e4:e4 + 2])
            # scalar queue: ctab, stab (needed by rope of tc0), lastmask
            ctab_sb = const.tile([D, T], BF16)
            nc.scalar.dma_start(out=ctab_sb, in_=ctab_d[:])
            stab_sb = const.tile([D, T], BF16)
            nc.scalar.dma_start(out=stab_sb, in_=stab_d[:])
            lastmask = const.tile([128, 512], BF16)
            nc.gpsimd.dma_start(out=lastmask, in_=lastmask_d[:])

            # persistent activation tensors
            kT = persist.tile([D, T], BF16)
            qT = persist.tile([D, N_SLOT, BLK], BF16)
            v_nat = persist.tile([128, T // 128, D], BF16)

            # q-table gather: slot s tables = permuted-ctab cols [512s, 512s+256)
            qctab = const.tile([D, N_SLOT * BLK], BF16)
            qstab = const.tile([D, N_SLOT * BLK], BF16)
            for s in range(N_SLOT):
                cs = slice(s * 512, s * 512 + BLK)
                ds = slice(s * BLK, (s + 1) * BLK)
                nc.gpsimd.dma_start(out=qctab[:, ds], in_=ctab_sb[:, cs])
                nc.gpsimd.dma_start(out=qstab[:, ds], in_=stab_sb[:, cs])

            # ---- PE warmup: ramp p-state while inputs stream in ----
            pwarm = ps.tile([128, 512], F32, tag="s", name="pwarm")
            for _ in range(7):
                nc.tensor.matmul(pwarm, ones, scratch, start=True, stop=True)

            qT_flat = qT.rearrange("p s b -> p (s b)")

            def kv_chunk(tci):
                cs = slice(tci * 512, (tci + 1) * 512)
                psk = pp.tile([128, 512], F32, tag="psk", name=f"psk{tci}")
                for ec in range(N_EC):
                    nc.tensor.matmul(psk, w_sb["k"][:, ec, :], xt[:, tci, ec, :],
                                     start=ec == 0, stop=ec == N_EC - 1)
                # V natural directly: out[t,d] = sum_e x[e,t] wv[e,d]
                # (xt 128-token chunk stationary, wv moving)
                psvT = pp.tile([128, 4, 128], F32, tag="psvT", name=f"psvT{tci}")
                for j in range(4):
                    for ec in range(N_EC):
                        nc.tensor.matmul(
                            psvT[:, j, :],
                            xt[:, tci, ec, j * 128:(j + 1) * 128],
                            w_sb["v"][:, ec, :],
                            start=ec == 0, stop=ec == N_EC - 1)
                # rope K: ACT copy psum->sbuf, gpsimd pair-swap, DVE combine
                raw = work.tile([128, 512], F32, tag="raw")
                nc.scalar.copy(raw, psk)
                sw = work.tile([128, 512], F32, tag="sw")
                s2 = raw.rearrange("(a b) f -> a b f", b=2)
                d2 = sw.rearrange("(a b) f -> a b f", b=2)
                nc.gpsimd.dma_start(out=d2[:, 0, :], in_=s2[:, 1, :])
                nc.gpsimd.dma_start(out=d2[:, 1, :], in_=s2[:, 0, :])
                t1 = work.tile([128, 512], F32, tag="ropeA")
                nc.vector.tensor_mul(t1, raw, ctab_sb[:, cs])
                t2 = work.tile([128, 512], F32, tag="ropeB")
                nc.vector.tensor_mul(t2, sw, stab_sb[:, cs])
                nc.vector.tensor_add(kT[:, cs], t1, t2)
                # evict V chunk to SBUF (bf16) on ACT
                vslice = v_nat[:, tci * 4:(tci + 1) * 4, :]
                nc.scalar.copy(vslice, psvT)

            def q_slot(s):
                qs_ = slice(s * BLK, (s + 1) * BLK)
                psq = pqp.tile([128, BLK], F32, tag="psq", name=f"psq{s}")
                for ec in range(N_EC):
                    nc.tensor.matmul(psq, w_sb["q"][:, ec, :],
                                     xt[:, s, ec, 0:BLK],
                                     start=ec == 0, stop=ec == N_EC - 1)
                qraw = work.tile([128, BLK], F32, tag="qraw")
                nc.scalar.copy(qraw, psq)
                qsw = work.tile([128, BLK], F32, tag="qsw")
                qs2 = qraw.rearrange("(a b) f -> a b f", b=2)
                qd2 = qsw.rearrange("(a b) f -> a b f", b=2)
                nc.gpsimd.dma_start(out=qd2[:, 0, :], in_=qs2[:, 1, :])
                nc.gpsimd.dma_start(out=qd2[:, 1, :], in_=qs2[:, 0, :])
                qt1 = work.tile([128, BLK], F32, tag="qropeA")
                nc.vector.tensor_mul(qt1, qraw, qctab[:, qs_])
                qt2 = work.tile([128, BLK], F32, tag="qropeB")
                nc.vector.tensor_mul(qt2, qsw, qstab[:, qs_])
                nc.vector.tensor_add(qT_flat[:, qs_], qt1, qt2)

            # ---- attention slot: software-pipelined S/exp vs D/AV ----
            def emit_S(s, p):
                pss = ps.tile([128, 512], F32, tag="s", name=f"pss{s}_{p}")
                ks = 256 * p
                nc.tensor.matmul(pss[:, 0:BLK], kT[:, ks:ks + 128],
                                 qT[:, s, :], start=True, stop=True)
                nc.tensor.matmul(pss[:, BLK:2 * BLK], kT[:, ks + 128:ks + 256],
                                 qT[:, s, :], start=True, stop=True)
                pT = work.tile([128, 512], BF16, tag="pT", bufs=4)
                nc.scalar.activation(out=pT, in_=pss,
                                     func=mybir.ActivationFunctionType.Exp,
                                     scale=float(SCALE))
                if p == 2 * s:  # own half: structural diagonal mask
                    nc.vector.tensor_mul(pT, pT, mask)
                elif p == 2 * s + 1:  # other half: per-core ones/zeros
                    nc.vector.tensor_mul(pT, pT, lastmask)
                return pT

            pacc = {}

            def emit_DAV(s, p, pT):
                np_ = 2 * (s + 1)
                if p == 0:
                    pacc_av = pa.tile([128, BLK], F32, tag="pacc_av", name=f"pav{s}")
                    pacc_d = pa.tile([128, BLK], F32, tag="pacc_d", name=f"pd{s}")
                    pacc[s] = (pacc_av, pacc_d)
                pacc_av, pacc_d = pacc[s]
                st, sp = p == 0, p == np_ - 1
                nc.tensor.matmul(pacc_d, ones, pT[:, 0:BLK], start=st, stop=False)
                nc.tensor.matmul(pacc_d, ones, pT[:, BLK:2 * BLK],
                                 start=False, stop=sp)
                nc.tensor.matmul(pacc_av, v_nat[:, 2 * p, :], pT[:, 0:BLK],
                                 start=st, stop=False)
                nc.tensor.matmul(pacc_av, v_nat[:, 2 * p + 1, :],
                                 pT[:, BLK:2 * BLK], start=False, stop=sp)
                if sp:  # slot done -> normalize + store
                    if dbg:
                        dcp = work.tile([128, BLK], F32, tag="dcp", name=f"dcp{s}")
                        nc.vector.tensor_copy(dcp, pacc_d)
                        nc.scalar.dma_start(out=den_o[:, s], in_=dcp)
                    recip = work.tile([128, BLK], F32, tag="recip")
                    nc.vector.reciprocal_approx_fast(recip, pacc_d)
                    oT = work.tile([128, BLK], F32, tag="oT")
                    nc.vector.tensor_mul(oT, pacc_av, recip)
                    nc.sync.dma_start(out=out_d[:, s], in_=oT)

            from collections import deque

            def attn_slot(s):
                pend = deque()
                for p in range(2 * (s + 1)):
                    pT = emit_S(s, p)
                    pend.append((s, p, pT))
                    if len(pend) > 2:
                        emit_DAV(*pend.popleft())
                while pend:
                    emit_DAV(*pend.popleft())

            # ---- schedule: rope-free Q projs fill the xt2 DMA window;
            # attn slots fill the xt3 window and rope-K3 latency ----
            kv_chunk(0)
            kv_chunk(1)
            q_slot(1)
            q_slot(0)
            kv_chunk(2)
            q_slot(2)
            attn_slot(2)
            kv_chunk(3)
            q_slot(3)
            attn_slot(1)
            attn_slot(0)
            attn_slot(3)

            if dbg:
                nc.scalar.dma_start(out=kT_o[:], in_=kT)
                nc.scalar.dma_start(out=qT_o[:], in_=qT)
                nc.scalar.dma_start(out=vn_o[:], in_=v_nat)

    nc.compile()
    return nc


_NC = None


def _get_nc():
    global _NC
    if _NC is None:
        _NC = _build_nc()
    return _NC


def _perm(h):
    """Permuted token order: within each 512-chunk, own 256 first."""
    own_off = 256 * (1 - h)  # h=0 owns odd blocks (cols 256:512 of chunk)
    idx = np.empty(T, dtype=np.int64)
    for s in range(N_TC):
        base = 512 * s
        idx[base:base + 256] = base + own_off + np.arange(256)
        idx[base + 256:base + 512] = base + (256 - own_off) + np.arange(256)
    return idx


def _host_prep(embedding_word, w_Q, w_K, w_V):
    x = np.asarray(embedding_word, dtype=np.float32)

    def pack_w(w):
        wt = np.asarray(w, dtype=np.float32).T.astype(BF16_NP)  # [E, D]
        return np.ascontiguousarray(wt.reshape(N_EC, 128, D).transpose(1, 0, 2))

    wq_p, wk_p, wv_p = pack_w(w_Q), pack_w(w_K), pack_w(w_V)

    j = np.arange(D // 2, dtype=np.float64)
    freqs = 1.0 / THETA ** (2.0 * j / D)
    t = np.arange(T, dtype=np.float64)
    ang = np.outer(freqs, t)
    cos = np.cos(ang)
    sin = np.sin(ang)
    ctab = np.repeat(cos, 2, axis=0).astype(BF16_NP)
    stab = np.empty((D, T), dtype=BF16_NP)
    stab[0::2] = -sin
    stab[1::2] = sin

    perms = {h: _perm(h) for h in (0, 1)}
    tabs = {h: (np.ascontiguousarray(ctab[:, perms[h]]),
                np.ascontiguousarray(stab[:, perms[h]])) for h in (0, 1)}
    lastmasks = {0: np.ones((128, 512), dtype=BF16_NP),
                 1: np.zeros((128, 512), dtype=BF16_NP)}

    xt_bh = {}
    for b in range(B):
        xTf = x[b].T.astype(BF16_NP)  # [E, T]
        for h in (0, 1):
            xT = xTf[:, perms[h]]
            xt_bh[b, h] = np.ascontiguousarray(
                xT.reshape(N_EC, 128, N_TC, 512).transpose(1, 2, 0, 3))

    in_maps = []
    for c in range(N_CORES):
        b, h = c // 2, c % 2
        in_maps.append({
            "xt": xt_bh[b, h],
            "wq": wq_p, "wk": wk_p, "wv": wv_p,
            "ctab": tabs[h][0], "stab": tabs[h][1],
            "lastmask": lastmasks[h],
        })
    return in_maps


def _assemble(results):
    out = np.empty((B, T, D), dtype=np.float32)
    for c in range(N_CORES):
        b, h = c // 2, c % 2
        o = results[c]["out"]  # [D, 4, 256] d-major; slot s = block 2s+(1-h)
        for s in range(N_SLOT):
            jb = 2 * s + (1 - h)
            out[b, jb * BLK:(jb + 1) * BLK, :] = o[:, s, :].T
    return out


def run(inputs, trace=False, tmpdir=None):
    nc = _get_nc()
    in_maps = _host_prep(**inputs)
    res = run_bass_kernel_spmd(nc, in_maps, list(range(N_CORES)),
                               trace=trace, tmpdir=tmpdir)
    return _assemble(res.results), res


def kernel(embedding_word, w_Q, w_K, w_V):
    out, _ = run(dict(embedding_word=embedding_word, w_Q=w_Q, w_K=w_K, w_V=w_V))
    return out
